# revision 52
# baseline (speedup 1.0000x reference)
"""Trainium2 Bass kernel for nn_EncoderLayer_71141838291874.

Transformer encoder layer, B=16 S=512 D=512 H=8 T=16 F=2048, with a
sinusoidal time-difference attention bias and the original's raw
reshape(B*H, S, DPH) head split.

Sharding: batch data-parallel over 8 cores; core c owns batches {c, c+8}
(same parity, so both share the same 8 time-bias matrices: jnp.tile maps
head-row bh to bias index bh % 16 = 8*(b%2) + h).

Key algebra:
 - t_bias[beta] = U @ V^T exactly (rank-2T via cos(x-y+p) expansion), fused
   into the QK^T matmul as 32 extra contraction rows (K = 64+32 = 96).
 - reshape(B*H,S,DPH) maps head h of batch b to rows [64h,64h+64) of Q viewed
   [S*8, 64]: s2 = s_local*8 + db (db = d//64, dph = d%64). Scores are
   computed with rows in "m-order" i = db*64+s_local and columns in true
   s2-order; the row permutation is absorbed into the attn DMA write pattern.
"""
import os
import numpy as np
import ml_dtypes

import concourse.bacc as bacc
import concourse.mybir as mybir
from concourse.tile import TileContext
from concourse.bass_utils import run_bass_kernel_spmd
from concourse.masks import make_identity

F32 = mybir.dt.float32
F32R = mybir.dt.float32r
BF16 = mybir.dt.bfloat16
AF = mybir.ActivationFunctionType
OP = mybir.AluOpType
AX = mybir.AxisListType
GELU_FN = None  # set below

B, S, D, H, T, F = 16, 512, 512, 8, 16, 2048
DPH = D // H               # 64
SCALE = (DPH // H) ** (-0.5)
LN_EPS = 1e-5
NCORES = 8
BPC = 2
P = 128
KD = D // P                # 4
NF = F // P                # 16
GELU_FN = AF.Identity if os.environ.get("KSIM") else AF.Gelu_apprx_tanh


def build_nc(skip_ln_affine=False, skip_b2=False):
    nc = bacc.Bacc("TRN2", target_bir_lowering=False)

    x_d = nc.dram_tensor("x", [BPC, S, D], F32, kind="ExternalInput")      # x + bo pre-added
    xt_d = nc.dram_tensor("xt", [BPC, D, S], F32R, kind="ExternalInput")
    wq_d = nc.dram_tensor("wq", [D, D], F32R, kind="ExternalInput")        # pre-scaled
    wk_d = nc.dram_tensor("wk", [D, D], F32R, kind="ExternalInput")
    wv_d = nc.dram_tensor("wv", [D, D], F32R, kind="ExternalInput")
    wo_d = nc.dram_tensor("wo", [D, D], BF16, kind="ExternalInput")
    w1_d = nc.dram_tensor("w1", [D, F], BF16, kind="ExternalInput")
    w2_d = nc.dram_tensor("w2", [F, D], BF16, kind="ExternalInput")
    bq_d = nc.dram_tensor("bqs", [D], F32, kind="ExternalInput")
    bk_d = nc.dram_tensor("bk", [D], F32, kind="ExternalInput")
    bv_d = nc.dram_tensor("bv", [D], F32, kind="ExternalInput")
    b1_d = nc.dram_tensor("b1f", [F], F32, kind="ExternalInput")
    b2_d = nc.dram_tensor("b2f", [D], F32, kind="ExternalInput")
    ln1g_d = nc.dram_tensor("ln1g", [D], F32, kind="ExternalInput")
    ln1b_d = nc.dram_tensor("ln1b", [D], F32, kind="ExternalInput")
    ln2g_d = nc.dram_tensor("ln2g", [D], F32, kind="ExternalInput")
    ln2b_d = nc.dram_tensor("ln2b", [D], F32, kind="ExternalInput")
    angu_d = nc.dram_tensor("ang_u", [H, 2 * T, S], F32, kind="ExternalInput")
    angv_d = nc.dram_tensor("ang_v", [H, 2 * T, S], F32, kind="ExternalInput")
    wt_d = nc.dram_tensor("u_wt", [2 * T], F32, kind="ExternalInput")

    attn_d = nc.dram_tensor("attn_part", [BPC * H, S, S], F32, kind="ExternalOutput")
    out2_d = nc.dram_tensor("out2_part", [BPC, S, D], F32, kind="ExternalOutput")

    with TileContext(nc) as tc:
        with (
            tc.tile_pool(name="wpool", bufs=1) as wpool,
            tc.tile_pool(name="qkt", bufs=1) as qkt,
            tc.tile_pool(name="xp", bufs=2) as xp,
            tc.tile_pool(name="soft", bufs=4) as soft,
            tc.tile_pool(name="atp", bufs=2) as atp,
            tc.tile_pool(name="vps", bufs=1) as vps,
            tc.tile_pool(name="ctxp", bufs=1) as ctxp,
            tc.tile_pool(name="ffp", bufs=1) as ffp,
            tc.tile_pool(name="smallp", bufs=3) as smallp,
            tc.tile_pool(name="pmain", bufs=3, space="PSUM") as pmain,
            tc.tile_pool(name="ptr", bufs=3, space="PSUM") as ptr,
            tc.tile_pool(name="pctx", bufs=2, space="PSUM") as pctx,
        ):
            # ================= constants =================
            wq_sb = [wpool.tile([P, D], F32R, tag=f"wq{k}", name=f"wq{k}") for k in range(KD)]
            wk_sb = [wpool.tile([P, D], F32R, tag=f"wk{k}", name=f"wk{k}") for k in range(KD)]
            wv_sb = [wpool.tile([P, D], F32R, tag=f"wv{k}", name=f"wv{k}") for k in range(KD)]
            wo_sb = [wpool.tile([P, D], BF16, tag=f"wo{k}", name=f"wo{k}") for k in range(KD)]
            w1_sb = [wpool.tile([P, F], BF16, tag=f"w1{k}", name=f"w1{k}") for k in range(KD)]
            w2_sb = [wpool.tile([P, D], BF16, tag=f"w2{k}", name=f"w2{k}") for k in range(NF)]
            for k in range(KD):
                nc.sync.dma_start(wq_sb[k][:], wq_d[k * P:(k + 1) * P, :])
                nc.sync.dma_start(wk_sb[k][:], wk_d[k * P:(k + 1) * P, :])
                nc.sync.dma_start(wv_sb[k][:], wv_d[k * P:(k + 1) * P, :])

            def bias_tile(dram, tag):
                t = wpool.tile([DPH, H], F32, tag=tag)
                nc.sync.dma_start(t[:], dram.rearrange("(db dph) -> dph db", dph=DPH))
                return t
            bq_sb = bias_tile(bq_d, "bq")
            bk_sb = bias_tile(bk_d, "bk")
            bv_sb = bias_tile(bv_d, "bv")

            def bcast_tile(dram, n, tag):
                t = wpool.tile([P, n], F32, tag=tag)
                nc.sync.dma_start(t[:], dram.rearrange("(o n) -> o n", o=1).to_broadcast((P, n)))
                return t
            b2_bc = None if skip_b2 else bcast_tile(b2_d, D, "b2_bc")
            if skip_ln_affine:
                g1_bc = be1_bc = g2_bc = be2_bc = None
            else:
                g1_bc = bcast_tile(ln1g_d, D, "g1_bc")
                be1_bc = bcast_tile(ln1b_d, D, "be1_bc")
                g2_bc = bcast_tile(ln2g_d, D, "g2_bc")
                be2_bc = bcast_tile(ln2b_d, D, "be2_bc")
            b1_sb = wpool.tile([P, NF], F32, tag="b1f", name="b1f")
            nc.sync.dma_start(b1_sb[:], b1_d.rearrange("(c p) -> p c", p=P))

            ident = wpool.tile([P, P], BF16, tag="ident", name="ident")
            make_identity(nc, ident)
            id32 = wpool.tile([P, P], F32, tag="id32", name="id32")
            make_identity(nc, id32)

            ones64 = wpool.tile([1, DPH], F32R, tag="ones64", name="ones64")
            nc.vector.memset(ones64[:].bitcast(F32), 1.0)
            wt_sb = wpool.tile([2 * T, 1], F32, tag="wt", name="wt")
            nc.sync.dma_start(wt_sb[:], wt_d.rearrange("(u o) -> u o", o=1))
            eps_sb = wpool.tile([P, 1], F32, tag="eps", name="eps")
            nc.vector.memset(eps_sb[:], LN_EPS)

            # ============ persistent QTr/KTr/Gv: [c, db, s] ============
            qt = qkt.tile([96, H, S], F32R, tag="QTr", name="QTr")
            kt = qkt.tile([96, H, S], F32R, tag="KTr", name="KTr")
            gv = qkt.tile([DPH, H, S], BF16, tag="Gv", name="Gv")

            # trig rows (once per core; QTr/KTr reused across both batches)
            for h in range(H):
                tsb = soft.tile([2 * T, S], F32, tag="a_norm", name="tsb")
                nc.sync.dma_start(tsb[:], angu_d[h])
                trig = soft.tile([2 * T, S], F32, tag="a_un", name="trig")
                nc.scalar.activation(trig[:], tsb[:], AF.Sin)
                nc.vector.tensor_scalar_mul(qt[64:96, h, :], trig[:], wt_sb[:])
                tsb2 = soft.tile([2 * T, S], F32, tag="a_norm", name="tsb2")
                nc.sync.dma_start(tsb2[:], angv_d[h])
                nc.scalar.activation(trig[:], tsb2[:], AF.Sin)
                nc.vector.tensor_copy(kt[64:96, h, :], trig[:])

            # late weight loads (not needed until Wo/FFN of batch 0)
            for k in range(KD):
                nc.sync.dma_start(wo_sb[k][:], wo_d[k * P:(k + 1) * P, :])
                nc.sync.dma_start(w1_sb[k][:], w1_d[k * P:(k + 1) * P, :])
            for k in range(NF):
                nc.sync.dma_start(w2_sb[k][:], w2_d[k * P:(k + 1) * P, :])

            # ================= per-batch pipeline =================
            for b in range(BPC):
                xt_sb = [xp.tile([P, S], F32R, tag=f"xt{k}", name=f"xt{k}") for k in range(KD)]
                for k in range(KD):
                    nc.sync.dma_start(xt_sb[k][:], xt_d[b, k * P:(k + 1) * P, :])
                x_sb = [xp.tile([P, D], F32, tag=f"x{u}", name=f"x{u}") for u in range(KD)]

                # ---- projections ----
                def project(w_sb, dest, bias, s2order):
                    # psum rows (a,dph); psum free s = h*64+sl
                    for mc in range(KD):
                        ps = pmain.tile([P, S], F32, tag="mm", name="mm")
                        for k in range(KD):
                            nc.tensor.matmul(ps[:], w_sb[k][:, mc * P:(mc + 1) * P],
                                             xt_sb[k][:], start=(k == 0), stop=(k == KD - 1))
                        for a in range(2):
                            db = 2 * mc + a
                            src_ap = ps[a * DPH:(a + 1) * DPH, :].rearrange(
                                "p (h sl) -> p h sl", h=H)
                            if s2order:
                                # dest free (h, s2=sl*8+db): [[h step 512],[sl step 8]] + db
                                dst_ap = dest[0:DPH, :, :].rearrange(
                                    "p h (sl db) -> p h sl db", db=H)[:, :, :, db:db + 1]
                            else:
                                # dest free (h, m=db*64+sl)
                                dst_ap = dest[0:DPH, :, db * DPH:(db + 1) * DPH]
                            if a == 0:
                                nc.vector.tensor_scalar_add(dst_ap, src_ap, bias[:, db:db + 1])
                            else:
                                nc.scalar.activation(dst_ap, src_ap, AF.Identity,
                                                     bias=bias[:, db:db + 1])
                project(wq_sb, qt, bq_sb, False)
                project(wk_sb, kt, bk_sb, True)
                project(wv_sb, gv, bv_sb, True)
                for u in range(KD):
                    nc.sync.dma_start(x_sb[u][:], x_d[b, u * P:(u + 1) * P, :])

                # ---- attention ----
                ctx_pairs = [ctxp.tile([P, KD * 2 * DPH], BF16, tag=f"cp{u}", name=f"cp{u}")
                             for u in range(KD)]
                for h in range(H):
                    # V' for this head: [j(s2), dph] bf16
                    vt = vps.tile([P, KD, DPH], BF16, tag="vp", name="vp")
                    for jc in range(KD):
                        tp = ptr.tile([P, DPH], BF16, tag="tp", name="tp")
                        in_ap = gv[:, h, P * jc:P * (jc + 1)]
                        nc.tensor.transpose(tp[:], in_ap, ident[0:DPH, 0:DPH])
                        nc.vector.tensor_copy(vt[:, jc, :], tp[:])
                    at_sb = atp.tile([P, KD, S], BF16, tag="at", name="at")
                    rinv_col = smallp.tile([P, KD], F32, tag="rc", name="rinv_col")
                    for t in range(KD):
                        # compute-critical first: AT_un = exp(K'Q') straight to bf16
                        ps2 = pmain.tile([P, S], F32, tag="mm", name="mm2")
                        nc.tensor.matmul(ps2[:], kt[:, h, P * t:P * (t + 1)], qt[:, h, :],
                                         start=True, stop=True)
                        nc.scalar.activation(at_sb[:, t, :], ps2[:], AF.Exp)
                        # DRAM-only path
                        ps = pmain.tile([P, S], F32, tag="mm", name="mm")
                        lhs = qt[:, h, P * t:P * (t + 1)]
                        rhs = kt[:, h, :]
                        nc.tensor.matmul(ps[:], lhs, rhs, start=True, stop=True)
                        a_un = soft.tile([P, S], F32, tag="a_un", name="a_un")
                        rsum = smallp.tile([P, 1], F32, tag="st1", name="rsum")
                        nc.scalar.activation(a_un[:], ps[:], AF.Exp, accum_out=rsum[:])
                        nc.vector.reciprocal(rinv_col[:, t:t + 1], rsum[:])
                        a_norm = soft.tile([P, S], F32, tag="a_norm", name="a_norm")
                        nc.gpsimd.tensor_scalar_mul(a_norm[:], a_un[:], rinv_col[:, t:t + 1])
                        # rows: partition p = 64*a2 + r -> attn row 8r + 2t + a2
                        dst_ap = attn_d[b * H + h].rearrange("(r e) j -> e r j", e=8)[
                            2 * t:2 * t + 2, :, :]
                        nc.sync.dma_start(dst_ap, a_norm[:])
                    # rinv broadcast row [64, 512]: transpose rinv_col then ones-matmul
                    rrow = vps.tile([1, S], F32R, tag="rrow", name="rrow")
                    for t in range(KD):
                        tpr = ptr.tile([1, P], F32, tag="tp", name="tpr")
                        nc.tensor.transpose(tpr[:], rinv_col[:, t:t + 1], id32[:])
                        nc.vector.tensor_copy(rrow[0:1, t * P:(t + 1) * P], tpr[:])
                    rb_ps = pctx.tile([DPH, S], F32, tag="ctx", name="rb_ps")
                    nc.tensor.matmul(rb_ps[:], ones64[:], rrow[:], start=True, stop=True)
                    rb_sb = vps.tile([DPH, S], F32, tag="rb", name="rb_sb")
                    nc.vector.tensor_copy(rb_sb[:], rb_ps[:])
                    cps = pctx.tile([DPH, S], F32, tag="ctx", name="ctx")
                    for jc in range(KD):
                        nc.tensor.matmul(cps[:], vt[:, jc, :], at_sb[:, jc, :],
                                         start=(jc == 0), stop=(jc == KD - 1))
                    # evict ctx psum [64 dph, i=(db,sl)] into packed [(a,dph), (db2,hh,sl)]
                    # normalizing by rinv[i] (rb_sb) on the way out
                    for a in range(2):
                        dst = ctx_pairs[h // 2][a * DPH:(a + 1) * DPH, :].rearrange(
                            "c (db2 hh sl) -> c db2 hh sl", db2=KD, hh=2)[:, :, h % 2, :]
                        src_ap = cps[:, :].rearrange("c (db sl) -> c db sl", db=H).rearrange(
                            "c (db2 a2) sl -> c db2 a2 sl", a2=2)[:, :, a, :]
                        rb_ap = rb_sb[:, :].rearrange("c (db sl) -> c db sl", db=H).rearrange(
                            "c (db2 a2) sl -> c db2 a2 sl", a2=2)[:, :, a, :]
                        nc.vector.tensor_tensor(dst, src_ap, rb_ap, op=OP.mult)

                # ---- Wo + residual (x already includes bo) ----
                out2a = [ffp.tile([P, D], F32, tag=f"o2a{u}", name=f"o2a{u}") for u in range(KD)]
                for u in range(KD):
                    po = pmain.tile([P, D], F32, tag="mm", name="mm")
                    for db2 in range(KD):
                        lhs = ctx_pairs[u][:, db2 * P:(db2 + 1) * P]
                        rhs = wo_sb[db2][:]
                        nc.tensor.matmul(po[:], lhs, rhs, start=(db2 == 0), stop=(db2 == KD - 1))
                    nc.vector.tensor_tensor(out2a[u][:], po[:], x_sb[u][:], op=OP.add)

                # ---- layernorm helper (in-place center on src) ----
                def layernorm(src_tiles, g_bc, be_bc, outs):
                    for u in range(KD):
                        xs = src_tiles[u]
                        s1 = smallp.tile([P, 1], F32, tag="st1", name="lns")
                        nc.vector.reduce_sum(s1[:], xs[:], axis=AX.X)
                        mneg = smallp.tile([P, 1], F32, tag="st2", name="lnm")
                        nc.vector.tensor_scalar_mul(mneg[:], s1[:], -1.0 / D)
                        nc.vector.tensor_scalar_add(xs[:], xs[:], mneg[:])
                        sq = ffp.tile([P, D], F32, tag="lnsq", name="lnsq")
                        ssq = smallp.tile([P, 1], F32, tag="st1", name="lnssq")
                        nc.scalar.activation(sq[:], xs[:], AF.Square, accum_out=ssq[:])
                        nc.vector.tensor_scalar_mul(ssq[:], ssq[:], 1.0 / D)
                        sd = smallp.tile([P, 1], F32, tag="st2", name="lnsd")
                        nc.scalar.activation(sd[:], ssq[:], AF.Sqrt, bias=eps_sb[:])
                        rstd = smallp.tile([P, 1], F32, tag="st3", name="lnr")
                        nc.vector.reciprocal(rstd[:], sd[:])
                        y = outs[u]
                        nc.scalar.activation(y[:], xs[:], AF.Copy, scale=rstd[:])
                        if not skip_ln_affine:
                            nc.vector.tensor_tensor(y[:], y[:], g_bc[:], op=OP.mult)
                            nc.vector.tensor_tensor(y[:], y[:], be_bc[:], op=OP.add)

                ln1 = [ffp.tile([P, D], F32, tag=f"lny{u}", name=f"lny{u}") for u in range(KD)]
                layernorm(out2a, g1_bc, be1_bc, ln1)

                # ---- LN1^T (bf16) via PE transpose ----
                ln1t = [ffp.tile([P, S], BF16, tag=f"ln1t{k}", name=f"ln1t{k}") for k in range(KD)]
                for k in range(KD):
                    for u in range(KD):
                        tp = ptr.tile([P, P], F32, tag="tp", name="tp")
                        nc.tensor.transpose(tp[:], ln1[u][:, k * P:(k + 1) * P], id32[:])
                        nc.vector.tensor_copy(ln1t[k][:, u * P:(u + 1) * P], tp[:])

                # ---- FFN1 -> gelu -> g1T bf16 ----
                g1t = [ffp.tile([P, S], BF16, tag=f"g1t{f}", name=f"g1t{f}") for f in range(NF)]
                for fc in range(NF):
                    pf = pmain.tile([P, S], F32, tag="mm", name="mm")
                    for k in range(KD):
                        nc.tensor.matmul(pf[:], w1_sb[k][:, fc * P:(fc + 1) * P], ln1t[k][:],
                                         start=(k == 0), stop=(k == KD - 1))
                    nc.scalar.activation(g1t[fc][:], pf[:], GELU_FN,
                                         bias=b1_sb[:, fc:fc + 1])

                # ---- FFN2 + residual + b2f ----
                ffo = out2a  # reuse slots (dead after LN1)
                for u in range(KD):
                    pf = pmain.tile([P, D], F32, tag="mm", name="mm")
                    for fc in range(NF):
                        nc.tensor.matmul(pf[:], g1t[fc][:, u * P:(u + 1) * P], w2_sb[fc][:],
                                         start=(fc == 0), stop=(fc == NF - 1))
                    nc.vector.tensor_tensor(ffo[u][:], pf[:], ln1[u][:], op=OP.add)
                    if not skip_b2:
                        nc.vector.tensor_tensor(ffo[u][:], ffo[u][:], b2_bc[:], op=OP.add)

                ln2 = [ffp.tile([P, D], F32, tag=f"lny{u}", name=f"ln2y{u}") for u in range(KD)]
                layernorm(ffo, g2_bc, be2_bc, ln2)
                for u in range(KD):
                    nc.sync.dma_start(out2_d[b, u * P:(u + 1) * P, :], ln2[u][:])

    nc.finalize()
    return nc


_NC_CACHE = None
LAST_EXEC_NS = None


def kernel(**inputs):
    global _NC_CACHE
    inp = {k: np.asarray(v) for k, v in inputs.items()}
    x = inp["inputs"].astype(np.float32)
    t_seq = inp["t_seq"].astype(np.float32)

    wq_s = (inp["Wq"] * np.float32(SCALE)).astype(np.float32)
    bq_s = (inp["bq"] * np.float32(SCALE)).astype(np.float32)
    tw = np.asarray(inp["time_w"], np.float32)
    tb = np.asarray(inp["time_b"], np.float32)
    twt = np.asarray(inp["time_weight"], np.float32)
    u_scale = np.repeat(tw, 2).astype(np.float32)
    u_bias = np.repeat(tb, 2).astype(np.float32)
    u_bias[0::2] += np.float32(np.pi / 2)
    v_bias = np.zeros(2 * T, np.float32)
    v_bias[0::2] = np.float32(np.pi / 2)
    u_wt = np.repeat(twt, 2).astype(np.float32)

    def wrap_pi(a):
        return (a + np.pi) % (2 * np.pi) - np.pi

    ivec = np.arange(S)
    s2_of_i = (ivec % 64) * 8 + (ivec // 64)

    x_res = x + np.asarray(inp["bo"], np.float32)[None, None, :]

    shared = dict(
        wq=wq_s, wk=np.asarray(inp["Wk"], np.float32),
        wv=np.asarray(inp["Wv"], np.float32),
        wo=np.asarray(inp["Wo"], np.float32).astype(ml_dtypes.bfloat16),
        w1=np.asarray(inp["W1"], np.float32).astype(ml_dtypes.bfloat16),
        w2=np.asarray(inp["W2"], np.float32).astype(ml_dtypes.bfloat16),
        bqs=bq_s, bk=np.asarray(inp["bk"], np.float32),
        bv=np.asarray(inp["bv"], np.float32),
        b1f=np.asarray(inp["b1f"], np.float32),
        b2f=np.asarray(inp["b2f"], np.float32),
        ln1g=np.asarray(inp["ln1_g"], np.float32),
        ln1b=np.asarray(inp["ln1_b"], np.float32),
        ln2g=np.asarray(inp["ln2_g"], np.float32),
        ln2b=np.asarray(inp["ln2_b"], np.float32),
        u_wt=u_wt,
    )

    in_maps = []
    for c in range(NCORES):
        bsel = [c, c + 8]
        p = c % 2
        ts_perm = np.stack([t_seq[8 * p + h][s2_of_i] for h in range(H)])   # [H, S] m-order
        ts_plain = t_seq[8 * p:8 * p + 8]                                   # [H, S] s2-order
        # angle tables, wrapped to [-pi, pi]
        ang_u = wrap_pi(u_scale[None, :, None] * ts_perm[:, None, :].astype(np.float64)
                        + u_bias[None, :, None]).astype(np.float32)
        ang_v = wrap_pi(u_scale[None, :, None] * ts_plain[:, None, :].astype(np.float64)
                        + v_bias[None, :, None]).astype(np.float32)
        m = dict(shared)
        m["x"] = np.ascontiguousarray(x_res[bsel])
        m["xt"] = np.ascontiguousarray(x[bsel].transpose(0, 2, 1))
        m["ang_u"] = np.ascontiguousarray(ang_u)
        m["ang_v"] = np.ascontiguousarray(ang_v)
        in_maps.append(m)

    if _NC_CACHE is None:
        skip_ln = (np.allclose(shared["ln1g"], 1) and np.allclose(shared["ln1b"], 0)
                   and np.allclose(shared["ln2g"], 1) and np.allclose(shared["ln2b"], 0))
        skip_b2 = bool(np.all(shared["b2f"] == 0))
        _NC_CACHE = build_nc(skip_ln_affine=skip_ln, skip_b2=skip_b2)
    global LAST_EXEC_NS
    if os.environ.get("KBENCH_TRACE"):
        try:
            br = run_bass_kernel_spmd(_NC_CACHE, in_maps, list(range(NCORES)), trace=True)
            LAST_EXEC_NS = br.exec_time_ns
            res = br.results
        except Exception:
            res = run_bass_kernel_spmd(_NC_CACHE, in_maps, list(range(NCORES))).results
    else:
        res = run_bass_kernel_spmd(_NC_CACHE, in_maps, list(range(NCORES))).results

    out2 = np.zeros((B, S, D), np.float32)
    attn = np.zeros((B * H, S, S), np.float32)
    for c in range(NCORES):
        out2[c] = res[c]["out2_part"][0]
        out2[c + 8] = res[c]["out2_part"][1]
        attn[8 * c:8 * c + 8] = res[c]["attn_part"][:8]
        attn[8 * (c + 8):8 * (c + 8) + 8] = res[c]["attn_part"][8:]
    return out2, attn
